# revision 14
# baseline (speedup 1.0000x reference)
"""Trainium2 Bass kernel for ragged bag-attention (nn_Attention).

Algorithm (per sentence i in bag b): logit_i = <x_i, att[q_i]*rel[q_i]>;
w = softmax(logit) within bag; out[b] = (sum_i w_i x_i) @ rel.T + bias.

Device strategy (8 cores, sentence-sharded, x shipped TRANSPOSED):
  - x rows packed into 128-row chunks; groups of GCH chunks share a PSUM
    accumulator with SLOTS bag-slots (bags may split across groups/cores;
    partial sums combined on host - exp(logit) is max-free safe, |logit|<~1).
  - x is sent d-major: 6 dtiles per chunk (dtiles 0-2 bf16 + a constant
    ones row, dtiles 3-5 fp8e4m3). PE computes, with the x-dtile as
    STATIONARY and bf16 [cw|rel|unit] moving (107 cols), the fused
    PY[s] = [P | Y | 1] = [x_s@cw.T | x_s@rel.T | 1] in f32 PSUM; 4 chunks
    share one PSUM bank tile.
  - logit = rowsum(onehot(q) * P) on DVE (53-wide affine_mul_reduce);
    onehot built on the idle Pool engine via is_equal vs an iota const.
  - ET[s,f] = exp(logit_s)*[slot_s==f] in ONE ACT op:
    exp(50*IND + logit), IND = [slot==f]-1 in {-1,0} from Pool is_equal.
  - [Y|1] copied to SBUF bf16 once per 4 chunks (strided 3D AP).
  - bag accum: PSUM[f,0:54] += ET.T @ [Y|1] (54 moving cols), flushed to
    SBUF every GCH chunks, one final DMA of the tiny U-table.
  - Host: num[bag] += U[slot,0:53], den[bag] += U[slot,53];
    out = num/den + bias.
"""
import sys
sys.path.insert(0, '/opt/trn_rl_repo')
import numpy as np

NCORES = 8
DIM = 690
NCLS = 53
CHUNK = 128
DW = 115            # dims per dtile (6*115 = 690)
XR = DW + 1         # bf16 x-tile rows: 115 data + a constant-ones row
NDT = 6
NBF = 3             # dtiles 0..2 bf16, dtiles 3..5 fp8e4m3
MOV = 2 * NCLS + 1  # 107 moving cols: [cw | rel | unit] -> PY = [P | Y | 1]
SLOTS = 64          # bag slots per PSUM group
GCH = 8             # chunks per PSUM group
DMAB = 16           # chunks per input DMA batch
PYC = 4             # chunks per PSUM PY tile / fused Y-copy
LAG = 6             # chunks between weight-build and bag matmul

_cache = {}         # nchunk -> compiled Bass module


def _pack_core(scope, seg, lo, hi):
    """Pack sentences [lo,hi) into 128-row chunks; groups of GCH chunks may
    hold at most SLOTS distinct bags (pad to group end when exceeded).
    Returns (rows, slots, f2b): sentence idx per row (-1 pad), slot per row,
    and per-group {bag: slot} maps."""
    group_rows = GCH * CHUNK
    rows, slots, f2b = [], [], []
    cur = None
    b0, b1 = int(seg[lo]), int(seg[hi - 1])
    for b in range(b0, b1 + 1):
        s = max(int(scope[b]), lo)
        e = min(int(scope[b + 1]), hi)
        while s < e:
            if len(rows) % group_rows == 0:
                cur = {}
                f2b.append(cur)
            gend = (len(rows) // group_rows + 1) * group_rows
            if b not in cur:
                if len(cur) == SLOTS:
                    pad = gend - len(rows)
                    rows.extend([-1] * pad)
                    slots.extend([-1] * pad)
                    continue
                cur[b] = len(cur)
            sl = cur[b]
            take = min(e - s, gend - len(rows))
            rows.extend(range(s, s + take))
            slots.extend([sl] * take)
            s += take
    return rows, slots, f2b


def _build_module(nchunk):
    from concourse import bacc, mybir
    from concourse.tile import TileContext

    f32 = mybir.dt.float32
    bf16 = mybir.dt.bfloat16
    fp8 = mybir.dt.float8e4
    eq = mybir.AluOpType.is_equal
    mult = mybir.AluOpType.mult
    sub = mybir.AluOpType.subtract
    ngroups = nchunk // GCH
    BW = NBF * CHUNK        # 384 cols per chunk in each stream
    assert nchunk % DMAB == 0 and nchunk % GCH == 0 and nchunk % PYC == 0

    nc = bacc.Bacc()
    xb_d = nc.declare_dram_parameter("xtb", [XR, nchunk * BW], bf16,
                                     isOutput=False)
    x8_d = nc.declare_dram_parameter("xt8", [DW, nchunk * BW], fp8,
                                     isOutput=False)
    qi_d = nc.declare_dram_parameter("qi", [CHUNK, nchunk], f32, isOutput=False)
    si_d = nc.declare_dram_parameter("si", [CHUNK, nchunk], f32, isOutput=False)
    io_d = nc.declare_dram_parameter("io", [CHUNK, SLOTS], bf16, isOutput=False)
    cw_d = nc.declare_dram_parameter("cwrel", [XR, NDT * MOV], bf16,
                                     isOutput=False)
    ut_d = nc.declare_dram_parameter("ut", [SLOTS, ngroups * 54], f32,
                                     isOutput=True)

    with TileContext(nc) as tc:
        with (
            tc.tile_pool(name="consts", bufs=1) as cpool,
            tc.tile_pool(name="xbb", bufs=3) as xbpool,
            tc.tile_pool(name="xb8", bufs=3) as x8pool,
            tc.tile_pool(name="oh", bufs=4) as ohpool,
            tc.tile_pool(name="ind", bufs=4) as indpool,
            tc.tile_pool(name="scr", bufs=2) as scrpool,
            tc.tile_pool(name="lg", bufs=4) as lgpool,
            tc.tile_pool(name="y", bufs=4) as ypool,
            tc.tile_pool(name="et", bufs=LAG + 2) as etpool,
            tc.tile_pool(name="py", bufs=3, space="PSUM") as pypool,
            tc.tile_pool(name="bag", bufs=2, space="PSUM") as bagpool,
        ):
            qi_sb = cpool.tile([CHUNK, nchunk], f32)
            nc.scalar.dma_start(out=qi_sb[:, :], in_=qi_d[:, :])
            si_sb = cpool.tile([CHUNK, nchunk], f32)
            nc.scalar.dma_start(out=si_sb[:, :], in_=si_d[:, :])
            io_sb = cpool.tile([CHUNK, SLOTS], bf16)
            nc.scalar.dma_start(out=io_sb[:, :], in_=io_d[:, :])
            cw_sb = cpool.tile([XR, NDT * MOV], bf16)
            nc.scalar.dma_start(out=cw_sb[:, :], in_=cw_d[:, :])
            ut_sb = cpool.tile([SLOTS, ngroups * 54], f32)

            ets, ys, bag = {}, {}, None

            def emit_bag(t2):
                nonlocal bag
                g, u = t2 // GCH, t2 % GCH
                if u == 0:
                    bag = bagpool.tile([SLOTS, 54], f32)
                yb4, uy = ys[t2]
                nc.tensor.matmul(bag[:, :], ets[t2],
                                 yb4[:, uy * 54:(uy + 1) * 54],
                                 start=(u == 0), stop=(u == GCH - 1))
                del ets[t2], ys[t2]
                if u == GCH - 1:
                    nc.scalar.copy(out=ut_sb[:, g * 54:(g + 1) * 54],
                                   in_=bag[:, :])

            xbb = xb8 = py = yb4 = None
            for t in range(nchunk):
                if t % DMAB == 0:
                    xbb = xbpool.tile([XR, DMAB * BW], bf16)
                    nc.sync.dma_start(
                        out=xbb[:, :],
                        in_=xb_d[:, t * BW:(t + DMAB) * BW])
                    xb8 = x8pool.tile([DW, DMAB * BW], fp8)
                    nc.scalar.dma_start(
                        out=xb8[:, :],
                        in_=x8_d[:, t * BW:(t + DMAB) * BW])
                xeb = xbb[:, (t % DMAB) * BW:]
                xe8 = xb8[:, (t % DMAB) * BW:]

                u4 = t % PYC
                if u4 == 0:
                    py = pypool.tile([CHUNK, PYC * MOV], f32)
                pys = py[:, u4 * MOV:(u4 + 1) * MOV]
                for j in range(NBF):
                    nc.tensor.matmul(
                        pys, xeb[:, j * CHUNK:(j + 1) * CHUNK],
                        cw_sb[:, j * MOV:(j + 1) * MOV],
                        start=(j == 0), stop=False)
                for j in range(NBF, NDT):
                    nc.tensor.matmul(
                        pys, xe8[:, (j - NBF) * CHUNK:(j - NBF + 1) * CHUNK],
                        cw_sb[0:DW, j * MOV:(j + 1) * MOV],
                        start=False, stop=(j == NDT - 1))

                oht = ohpool.tile([CHUNK, NCLS], bf16)
                nc.gpsimd.tensor_scalar(
                    out=oht[:, :], in0=io_sb[:, 0:NCLS],
                    scalar1=qi_sb[:, t:t + 1], scalar2=1.0, op0=eq, op1=mult)
                ind = indpool.tile([CHUNK, SLOTS], bf16)
                nc.gpsimd.tensor_scalar(
                    out=ind[:, :], in0=io_sb[:, :],
                    scalar1=si_sb[:, t:t + 1], scalar2=1.0, op0=eq, op1=sub)

                scr = scrpool.tile([CHUNK, NCLS], bf16)
                lg = lgpool.tile([CHUNK, 1], f32)
                nc.vector.affine_mul_reduce(
                    out=scr[:, :], accum_out=lg[:, :], in0=oht[:, :],
                    in1=pys[:, 0:NCLS], scale=1.0, bias=0.0)

                if u4 == PYC - 1:
                    yb4 = ypool.tile([CHUNK, PYC * 54], bf16)
                    nc.vector.tensor_copy(
                        out=yb4[:, :].rearrange("p (u c) -> p u c", u=PYC),
                        in_=py[:, :].rearrange("p (u c) -> p u c", u=PYC)
                            [:, :, NCLS:MOV])
                    for tt in range(t - PYC + 1, t + 1):
                        ys[tt] = (yb4, tt % PYC)

                et = etpool.tile([CHUNK, SLOTS], bf16)
                nc.scalar.activation(et[:, :], ind[:, :],
                                     mybir.ActivationFunctionType.Exp,
                                     bias=lg[:, 0:1], scale=50.0)
                ets[t] = et

                if t >= LAG:
                    emit_bag(t - LAG)
            for t2 in range(nchunk - LAG, nchunk):
                emit_bag(t2)

            nc.scalar.dma_start(out=ut_d[:, :], in_=ut_sb[:, :])

    nc.compile()
    return nc


def _prepare(x, rel_weight, att_weight, bias, attention_query, scope):
    import ml_dtypes
    x = np.asarray(x, dtype=np.float32)
    rel_weight = np.asarray(rel_weight, dtype=np.float32)
    att_weight = np.asarray(att_weight, dtype=np.float32)
    bias = np.asarray(bias, dtype=np.float32)
    q = np.asarray(attention_query).astype(np.int64)
    scope = np.asarray(scope).astype(np.int64)

    nsent = x.shape[0]
    nbags = len(scope) - 1
    score = nsent // NCORES
    seg = np.searchsorted(scope, np.arange(nsent), side='right') - 1

    packs = [_pack_core(scope, seg, c * score, (c + 1) * score)
             for c in range(NCORES)]
    nchunk = max((len(p[0]) + CHUNK - 1) // CHUNK for p in packs)
    lcm = int(np.lcm.reduce([GCH, DMAB, PYC]))
    nchunk = (nchunk + lcm - 1) // lcm * lcm
    S = nchunk * CHUNK
    ngroups = nchunk // GCH
    BW = NBF * CHUNK

    # [cw | rel | unit] blocked per dtile: [116, 6*107]; row 115 is the
    # constant-ones row of xtb, col 106 of dtile 0 routes it to PY[:,106]=1.
    cw = att_weight * rel_weight
    M = np.concatenate([cw, rel_weight], axis=0)        # [106, 690]
    cwrel = np.zeros((XR, NDT * MOV), np.float32)
    for j in range(NDT):
        cwrel[0:DW, j * MOV:j * MOV + 2 * NCLS] = M[:, j * DW:(j + 1) * DW].T
    cwrel[DW, 0 * MOV + 2 * NCLS] = 1.0
    cwrel = cwrel.astype(ml_dtypes.bfloat16)
    iot = np.ascontiguousarray(np.broadcast_to(
        np.arange(SLOTS, dtype=np.float32), (CHUNK, SLOTS))
    ).astype(ml_dtypes.bfloat16)

    in_maps, frag2bag = [], []
    for c in range(NCORES):
        rows, slots, f2b = packs[c]
        idx = np.full(S, -1, np.int64)
        idx[:len(rows)] = rows
        sl = np.full(S, -1, np.int64)
        sl[:len(slots)] = slots
        valid = idx >= 0

        xp = np.zeros((S, DIM), np.float32)
        xp[valid] = x[idx[valid]]
        xq = xp.reshape(nchunk, CHUNK, NDT, DW)
        # bf16 stream: dtiles 0..2 -> [115, nchunk, 3, 128] + ones row
        xtb = np.empty((XR, nchunk * BW), ml_dtypes.bfloat16)
        xtb[0:DW] = np.ascontiguousarray(
            xq[:, :, 0:NBF].astype(ml_dtypes.bfloat16).transpose(3, 0, 2, 1)
        ).reshape(DW, nchunk * BW)
        xtb[DW] = 1.0
        # fp8 stream: dtiles 3..5
        xt8 = np.ascontiguousarray(
            xq[:, :, NBF:NDT].astype(ml_dtypes.float8_e4m3fn)
            .transpose(3, 0, 2, 1)).reshape(DW, nchunk * BW)

        qp = np.full(S, -1.0, np.float32)
        qp[valid] = q[idx[valid]]
        si = sl.astype(np.float32)

        f2b_arr = np.full((ngroups, SLOTS), -1, np.int64)
        for g, m in enumerate(f2b):
            for b, s_ in m.items():
                f2b_arr[g, s_] = b
        frag2bag.append(f2b_arr)
        in_maps.append({
            "xtb": xtb,
            "xt8": xt8,
            "qi": np.ascontiguousarray(qp.reshape(nchunk, CHUNK).T),
            "si": np.ascontiguousarray(si.reshape(nchunk, CHUNK).T),
            "io": iot,
            "cwrel": cwrel,
        })
    return in_maps, frag2bag, nchunk, nbags, bias


def _assemble(tables, frag2bag, nchunk, nbags, bias):
    ngroups = nchunk // GCH
    num = np.zeros((nbags, NCLS))
    den = np.zeros(nbags)
    for c in range(NCORES):
        ut = np.asarray(tables[c], dtype=np.float64).reshape(
            SLOTS, ngroups, 54).transpose(1, 0, 2)   # [g, slot, 54]
        fb = frag2bag[c].ravel()
        U = ut.reshape(ngroups * SLOTS, 54)
        v = fb >= 0
        np.add.at(num, fb[v], U[v, 0:53])
        np.add.at(den, fb[v], U[v, 53])
    return (num / den[:, None] + bias[None, :]).astype(np.float32)


def kernel(x, rel_weight, att_weight, bias, attention_query, scope):
    from concourse.bass_utils import run_bass_kernel_spmd

    in_maps, frag2bag, nchunk, nbags, b = _prepare(
        x, rel_weight, att_weight, bias, attention_query, scope)
    if nchunk not in _cache:
        _cache[nchunk] = _build_module(nchunk)
    nc = _cache[nchunk]
    res = run_bass_kernel_spmd(nc, in_maps, list(range(NCORES)))
    tables = [res.results[c]["ut"] for c in range(NCORES)]
    return _assemble(tables, frag2bag, nchunk, nbags, b)
